# revision 4
# baseline (speedup 1.0000x reference)
"""Causal self-attention (B=2,T=2048,C=1024,H=16,hd=64) with QK-RMSNorm + RoPE.

8-core Trainium2 Bass kernel, tensor-parallel over heads (2 heads per core).

Host<->device traffic is the bottleneck in this harness (per-dispatch
resharding of every operand array), so the kernel ships ONE packed bf16
tensor per core (~2.5MB) and one bf16 output (1MB):
  - x is token-sharded on the wire (512 tokens/core, feature-major) and
    AllGathered on-device; w_attn is row-sharded (core's 2 heads); w_proj is
    column-sharded (core's 128 head-features); cos/sin tables ship once at
    [32/64, T] and are replicated across partitions on-device; masks ship
    bf16 and are widened on-device; 0/1 selector constants are memset.
  - c_proj: each core computes partial (its 128 features x full w_proj rows)
    for ALL tokens, then a ReduceScatter(add) leaves each core the exact
    final output for its 512-token slice, shipped back bf16 feature-major.

On-device layout strategy (unchanged from the tuned baseline): everything
feature-major; q,k feature order permuted to [evens, odds] per head so
interleaved RoPE becomes half-block ops; softmax denominator via a
ones-column appended to V (|s| <= 8 after RMS-norm, exp needs no max).
"""

import numpy as np

import concourse.bass as bass
import concourse.mybir as mybir
import concourse.tile as tile
from concourse import bacc
from concourse.bass_utils import run_bass_kernel_spmd

B, T, C = 2, 2048, 1024
H, HD = 16, 64
N_CORES = 8
HPC = H // N_CORES  # heads per core = 2
BT = B * T  # 4096 flattened tokens
FPC = HPC * HD  # feats per core = 128
EPS = 1e-6
TN = BT // 512  # 8 token tiles of 512
QB = T // 512  # 4 query blocks per sequence

f32 = mybir.dt.float32
f32r = mybir.dt.float32r
bf16 = mybir.dt.bfloat16
MUL = mybir.AluOpType.mult
ADD = mybir.AluOpType.add
AF = mybir.ActivationFunctionType

RG = [list(range(N_CORES))]

# packed-input regions (element offsets into the flat bf16 "inp" tensor)
_off = 0
_REG = {}
for _name, _n in (
    ("X", C * 512),        # xT slice [C, 512] feature-major, core's tokens
    ("WA", C * 3 * FPC),   # waT [C, 384] = w_attn[sel_rows].T
    ("WP", FPC * C),       # wpTc [128, 1024] = w_proj[:, fc].T
    ("CS", 32 * T),        # cos.T [32, T]
    ("SN2", 64 * T),       # [-sin; sin].T [64, T]
    ("ID", 128 * 128),     # identity
    ("WG", 128 * 128),     # causal wedge (i <= j)
    ("QK", 128 * 2),       # col0 = q_norm_w[perm] tiled, col1 = k_norm_w
):
    _REG[_name] = (_off, _n)
    _off += _n
IN_ELEMS = _off


def r32(ap):
    return ap.bitcast(f32r)


def build_nc(single_core=False, no_cc=False):
    no_cc = no_cc or single_core
    nc = bacc.Bacc("TRN2", target_bir_lowering=False, debug=False,
                   num_devices=1 if single_core else N_CORES)

    inp = nc.dram_tensor("inp", [IN_ELEMS], bf16, kind="ExternalInput")
    out = nc.dram_tensor("out", [C, 512], bf16, kind="ExternalOutput")

    def region(name):
        off, n = _REG[name]
        return inp.ap()[off:off + n]

    with tile.TileContext(nc) as tc:
        with (
            tc.tile_pool(name="const", bufs=1) as const,
            tc.tile_pool(name="resid", bufs=1) as resid,
            tc.tile_pool(name="xtp", bufs=6) as xtp,
            tc.tile_pool(name="work", bufs=3) as work,
            tc.tile_pool(name="pwork", bufs=4) as pwork,
            tc.tile_pool(name="mm", bufs=2, space="PSUM") as mmp,
            tc.tile_pool(name="yp", bufs=2, space="PSUM") as ypp,
            tc.tile_pool(name="sp", bufs=1, space="PSUM") as spp,
            tc.tile_pool(name="bcp", bufs=1, space="PSUM") as bcp,
            tc.tile_pool(name="dram", bufs=1, space="DRAM") as dramp,
        ):
            # ---- x AllGather (token-sharded on the wire) ----
            ag_in = dramp.tile([C * 512], bf16, tag="ag_in")
            nc.sync.dma_start(ag_in[:], region("X"))
            ag_out = dramp.tile([N_CORES, C * 512], bf16, tag="ag_out")
            if no_cc:
                for r in range(N_CORES):
                    nc.sync.dma_start(ag_out[r], ag_in[:])
            else:
                nc.gpsimd.collective_compute(
                    "AllGather", mybir.AluOpType.bypass, replica_groups=RG,
                    ins=[ag_in[:].opt()], outs=[ag_out[:, :].opt()])

            # ---- constants to SBUF ----
            wa_sb = const.tile([128, C // 128, 3 * FPC], bf16, tag="wa")
            nc.sync.dma_start(
                wa_sb[:], region("WA").rearrange("(o p f) -> p o f",
                                                 p=128, f=3 * FPC))
            wp_sb = const.tile([128, C], bf16, tag="wp")
            nc.sync.dma_start(wp_sb[:], region("WP").rearrange("(p f) -> p f",
                                                               f=C))
            qk_bf = const.tile([128, 2], bf16, tag="qkb")
            nc.sync.dma_start(qk_bf[:], region("QK").rearrange("(p c) -> p c",
                                                               c=2))
            qk_sb = const.tile([128, 2], f32, tag="qkf")
            nc.vector.tensor_copy(qk_sb[:], qk_bf[:])
            qw_sb = qk_sb[:, 0:1]
            kw_sb = qk_sb[:, 1:2]

            # 0/1 selector constants via memset
            bo_sb = const.tile([128, 2], f32, tag="bo")
            nc.vector.memset(bo_sb[0:64, 0:1], 1.0)
            nc.vector.memset(bo_sb[64:128, 0:1], 0.0)
            nc.vector.memset(bo_sb[0:64, 1:2], 0.0)
            nc.vector.memset(bo_sb[64:128, 1:2], 1.0)
            # s2 = bo^T; transpose via DRAM roundtrip (SBUF free dims can't
            # cross partitions, and engines can't address partition 1 alone)
            s2_sb = const.tile([2, 128], f32, tag="s2")
            bo_dr = dramp.tile([128, 2], f32, tag="bo_dr")
            nc.sync.dma_start(bo_dr[:, :], bo_sb[:, :])
            nc.sync.dma_start(s2_sb[:, :], bo_dr[:, :].rearrange("p c -> c p"))
            eps_sb = const.tile([128, 1], f32, tag="eps")
            nc.vector.memset(eps_sb[:], EPS)
            ones_sb = const.tile([128, 32], f32, tag="ones")
            nc.vector.memset(ones_sb[:], 1.0)

            cs_sb = const.tile([128, T], f32, tag="cs")
            sn_sb = const.tile([128, T], f32, tag="sn")
            id_sb = const.tile([128, 128], f32, tag="id")
            wg_sb = const.tile([128, 128], f32, tag="wg")

            def emit_late_consts():
                # ones columns of V (softmax denominator)
                nc.sync.dma_start(vA[:, :, HD], r32(ones_sb[:]))
                nc.sync.dma_start(vA[:, :, 2 * HD + 1], r32(ones_sb[:]))
                # cos/sin: replicate [32/64, T] bf16 regions across partitions
                cs_bf = work.tile([128, T], bf16, tag="csb", name="cs_bf")
                csr = region("CS").rearrange("(p t) -> p t", t=T)
                for k in range(4):
                    nc.sync.dma_start(cs_bf[32 * k:32 * k + 32, :], csr)
                nc.vector.tensor_copy(cs_sb[:], cs_bf[:])
                sn_bf = work.tile([128, T], bf16, tag="csb", name="sn_bf")
                snr = region("SN2").rearrange("(p t) -> p t", t=T)
                for k in range(2):
                    nc.sync.dma_start(sn_bf[64 * k:64 * k + 64, :], snr)
                nc.vector.tensor_copy(sn_sb[:], sn_bf[:])
                # identity / causal wedge
                m_bf = work.tile([128, 128], bf16, tag="mkb", name="id_bf")
                nc.sync.dma_start(m_bf[:], region("ID").rearrange(
                    "(p f) -> p f", f=128))
                nc.vector.tensor_copy(id_sb[:], m_bf[:])
                m_bf2 = work.tile([128, 128], bf16, tag="mkb", name="wg_bf")
                nc.sync.dma_start(m_bf2[:], region("WG").rearrange(
                    "(p f) -> p f", f=128))
                nc.vector.tensor_copy(wg_sb[:], m_bf2[:])

            # ---- residents ----
            qT = resid.tile([128, BT], f32r, tag="qT")   # roped+normed q^T
            kT = resid.tile([128, BT], f32r, tag="kT")
            # attention out^T, both heads packed [128, BT]; written via
            # SBUF->SBUF DMA (cross-partition moves are DMA-only)
            yHp = resid.tile([128, BT], bf16, tag="yHp")
            # V in token-major + ones cols: per head h: cols [65h:65h+64]=V_h,
            # col 65h+64 = 1.0
            vA = resid.tile([128, BT // 128, 2 * (HD + 1)], f32r, tag="vA")

            # ================= QKV + RMSNorm + RoPE =================
            xts = {}

            def emit_xt(n):
                xn = ag_out[n].rearrange("(o p t) -> p o t", p=128, t=512)
                xtA = xtp.tile([128, 4, 512], bf16, tag="xt", name=f"xtA{n}")
                nc.sync.dma_start(xtA[:], xn[:, 0:4, :])
                xtB = xtp.tile([128, 4, 512], bf16, tag="xt", name=f"xtB{n}")
                nc.sync.dma_start(xtB[:], xn[:, 4:8, :])
                xts[n] = (xtA, xtB)

            def emit_qkv(n):
                tok = slice(512 * n, 512 * n + 512)
                ct = slice(512 * (n % 4), 512 * (n % 4) + 512)
                if n not in xts:
                    emit_xt(n)
                xtA, xtB = xts.pop(n)

                bigQK = mmp.tile([128, 1024], f32, tag="big", name=f"qk{n}")
                bigV = mmp.tile([128, 1024], f32, tag="big", name=f"v{n}")
                for m, dst, wcol in ((0, qT, qw_sb), (1, kT, kw_sb), (2, None, None)):
                    ps = bigV[:, 0:512] if m == 2 else bigQK[:, 512 * m:512 * m + 512]
                    for kt in range(C // 128):
                        nc.tensor.matmul(
                            ps,
                            wa_sb[:, kt, 128 * m:128 * m + 128],
                            xtA[:, kt, :] if kt < 4 else xtB[:, kt - 4, :],
                            start=(kt == 0), stop=(kt == C // 128 - 1),
                        )
                    if m == 2:
                        # V: token-major via PE transpose of 128x128 blocks
                        vs = work.tile([128, 512], f32, tag="vs", name=f"vs{n}")
                        nc.scalar.copy(vs[:], ps)
                        for j in range(4):
                            pt = spp.tile([128, 128], f32, tag="sm", name=f"vt{n}_{j}")
                            nc.tensor.transpose(pt[:], vs[:, 128 * j:128 * j + 128],
                                                id_sb[:])
                            kt_g = 4 * n + j
                            nc.vector.tensor_copy(
                                vA[:, kt_g].rearrange("p (h d) -> p h d", h=2)[:, :, 0:HD],
                                pt[:, :].rearrange("p (h d) -> p h d", h=2))
                        continue

                    # stats from raw (pre-weight) psum
                    sq = work.tile([128, 512], f32, tag="scr", name=f"sq{n}_{m}")
                    nc.scalar.activation(r32(sq[:]), ps, AF.Square)
                    ss = spp.tile([2, 512], f32, tag="sm", name=f"ss{n}_{m}")
                    nc.tensor.matmul(ss[:], r32(bo_sb[:]), r32(sq[:]),
                                     start=True, stop=True)
                    inv = work.tile([2, 512], f32, tag="rms", name=f"rms{n}_{m}")
                    nc.scalar.activation(r32(inv[:]), ss[:], AF.Sqrt,
                                         bias=eps_sb[0:2, :], scale=1.0 / HD)
                    with nc.allow_low_precision(reason="f32r is fp32-width"):
                        nc.vector.reciprocal(r32(inv[:]), inv[:])

                    # apply norm weight on the way out of PSUM
                    nc.vector.tensor_scalar_mul(dst[:, tok], ps, wcol)

                    # rope: r = q*CS + swap(q)*SN  (swap halves within head)
                    sw = work.tile([128, 512], f32r, tag="sw", name=f"sw{n}_{m}")
                    for h in range(HPC):
                        b0 = 64 * h
                        nc.sync.dma_start(sw[b0:b0 + 32, :], dst[b0 + 32:b0 + 64, tok])
                        nc.sync.dma_start(sw[b0 + 32:b0 + 64, :], dst[b0:b0 + 32, tok])
                    nc.gpsimd.tensor_tensor(sw[:], sw[:], sn_sb[:, ct], MUL)
                    nc.vector.tensor_tensor(dst[:, tok], dst[:, tok], cs_sb[:, ct], MUL)
                    nc.vector.tensor_tensor(dst[:, tok], dst[:, tok], sw[:], ADD)

                    # apply 1/rms: broadcast [2,512] -> [128,512] via K=2 matmul
                    bc = bcp.tile([128, 512], f32, tag="bc", name=f"bc{n}_{m}")
                    nc.tensor.matmul(bc[:], r32(s2_sb[:]), r32(inv[:]),
                                     start=True, stop=True)
                    nc.vector.tensor_tensor(r32(dst[:, tok]), dst[:, tok], bc[:], MUL)

            # ================= causal attention =================
            def emit_attn(b, i):
                if True:
                    qcol = slice(2048 * b + 512 * i, 2048 * b + 512 * i + 512)
                    nkt = 4 * i + 4
                    yps = [ypp.tile([HD + 1, 512], f32, tag="y",
                                    name=f"y{b}_{i}_{h}") for h in range(HPC)]
                    for kt in range(nkt):
                        qs = 128 * (kt - 4 * i) if kt >= 4 * i else 0
                        kc = 2048 * b + 128 * kt
                        kt_g = 16 * b + kt
                        sps = mmp.tile([128, 1024], f32, tag="big",
                                       name=f"s{b}_{i}_{kt}")
                        pT = pwork.tile([128, 1024], f32, tag="pT",
                                        name=f"p{b}_{i}_{kt}")
                        for h in range(HPC):
                            hb = 64 * h
                            nc.tensor.matmul(
                                sps[:, 512 * h + qs:512 * h + 512],
                                r32(kT[hb:hb + 64, kc:kc + 128]),
                                r32(qT[hb:hb + 64, qcol][:, qs:]),
                                start=True, stop=True,
                                tile_position=(hb, 0),
                            )
                        sps3 = sps[:, :].rearrange("p (h q) -> p h q", h=2)[:, :, qs:]
                        pT3 = pT[:, :].rearrange("p (h q) -> p h q", h=2)[:, :, qs:]
                        nc.scalar.activation(r32(pT3), sps3, AF.Exp,
                                             scale=1.0 / 8.0)
                        for h in range(HPC):
                            if kt >= 4 * i:
                                nc.gpsimd.tensor_tensor(
                                    r32(pT[:, 512 * h + qs:512 * h + qs + 128]),
                                    pT[:, 512 * h + qs:512 * h + qs + 128],
                                    wg_sb[:], MUL)
                            nc.tensor.matmul(
                                yps[h][:, qs:],
                                r32(vA[:, kt_g, (HD + 1) * h:(HD + 1) * h + HD + 1]),
                                r32(pT[:, 512 * h + qs:512 * h + 512]),
                                start=(kt == 0), stop=(kt == nkt - 1),
                            )
                    # normalize by the ones-column denominator
                    for h in range(HPC):
                        di = work.tile([1, 512], f32, tag="rms",
                                       name=f"di{b}_{i}_{h}")
                        with nc.allow_low_precision(reason="f32r is fp32-width"):
                            nc.vector.reciprocal(r32(di[:]), yps[h][HD:HD + 1, :])
                        dp = spp.tile([64, 512], f32, tag="sm",
                                      name=f"dp{b}_{i}_{h}")
                        nc.tensor.matmul(dp[:], r32(s2_sb[0:1, 0:64]), r32(di[:]),
                                         start=True, stop=True)
                        dpS = work.tile([64, 512], f32, tag="dpS",
                                        name=f"dpS{b}_{i}_{h}")
                        nc.scalar.copy(dpS[:], dp[:])
                        ybf = work.tile([HD, 512], bf16, tag="ybf",
                                        name=f"ybf{b}_{i}_{h}")
                        nc.vector.tensor_tensor(ybf[:, :],
                                                yps[h][:HD, :], dpS[:, :],
                                                MUL)
                        nc.sync.dma_start(yHp[64 * h:64 * h + HD, qcol],
                                          ybf[:, :])

            emit_xt(0)
            emit_late_consts()
            emit_qkv(0)
            for n in range(1, TN // 2):
                emit_qkv(n)
                emit_attn(0, n - 1)
            emit_qkv(TN // 2)
            emit_attn(0, 3)
            for n in range(TN // 2 + 1, TN):
                emit_qkv(n)
                emit_attn(1, n - TN // 2 - 1)
            emit_attn(1, 3)

            # ======== c_proj partials (all tokens) + ReduceScatter ========
            rs_in = dramp.tile([N_CORES, C, 512], bf16, tag="rs_in")
            for ob in range(8):
                obl = slice(128 * ob, 128 * ob + 128)
                for tg in range(4):
                    bigP = mmp.tile([128, 1024], f32, tag="big",
                                    name=f"po{ob}_{tg}")
                    for hh in range(2):
                        tt = 2 * tg + hh
                        nc.tensor.matmul(
                            bigP[:, 512 * hh:512 * hh + 512],
                            wp_sb[:, obl],
                            yHp[:, 512 * tt:512 * tt + 512],
                            start=True, stop=True)
                    ob_sb = work.tile([128, 1024], bf16, tag="obp",
                                      name=f"ob{ob}_{tg}")
                    nc.scalar.copy(ob_sb[:], bigP[:])
                    for hh in range(2):
                        tt = 2 * tg + hh
                        nc.sync.dma_start(rs_in[tt, obl, :],
                                          ob_sb[:, 512 * hh:512 * hh + 512])
            rs_out = dramp.tile([C, 512], bf16, tag="rs_out")
            if no_cc:
                nc.sync.dma_start(rs_out[:, :], rs_in[0])
            else:
                nc.gpsimd.collective_compute(
                    "ReduceScatter", ADD, replica_groups=RG,
                    ins=[rs_in[:, :, :].opt()], outs=[rs_out[:, :].opt()])
            nc.sync.dma_start(out.ap(), rs_out[:, :])

    nc.compile()
    return nc


def make_in_maps(x, freqs_cos, freqs_sin, w_attn, w_proj, q_norm_w, k_norm_w):
    import ml_dtypes
    x = np.asarray(x, np.float32)
    freqs_cos = np.asarray(freqs_cos, np.float32)
    freqs_sin = np.asarray(freqs_sin, np.float32)
    w_attn = np.asarray(w_attn, np.float32)
    w_proj = np.asarray(w_proj, np.float32)
    q_norm_w = np.asarray(q_norm_w, np.float32)
    k_norm_w = np.asarray(k_norm_w, np.float32)

    perm = np.concatenate([np.arange(0, HD, 2), np.arange(1, HD, 2)])
    xTf = np.ascontiguousarray(x.reshape(BT, C).T)  # [C, BT] f32

    cs = freqs_cos.T  # [32, T]
    sn = freqs_sin.T
    sn2 = np.concatenate([-sn, sn], axis=0)  # [64, T]
    ident = np.eye(128, dtype=np.float32)
    wedge = (np.arange(128)[:, None] <= np.arange(128)[None, :]).astype(
        np.float32)
    qkw = np.stack([np.tile(q_norm_w[perm], HPC),
                    np.tile(k_norm_w[perm], HPC)], axis=1)  # [128, 2]

    in_maps = []
    for c in range(N_CORES):
        rows = []
        for sec in range(3):  # q, k, v sections of w_attn
            for h in (HPC * c, HPC * c + 1):
                base = C * sec + HD * h
                if sec < 2:
                    rows.append(base + perm)
                else:
                    rows.append(base + np.arange(HD))
        sel_rows = np.concatenate(rows)
        waT = w_attn[sel_rows].T  # [C, 384]
        wpTc = w_proj[:, FPC * c:FPC * c + FPC].T  # [128, 1024]
        xc = xTf[:, 512 * c:512 * c + 512]  # [C, 512]

        packed = np.concatenate([
            xc.ravel(), waT.ravel(), wpTc.ravel(), cs.ravel(), sn2.ravel(),
            ident.ravel(), wedge.ravel(), qkw.ravel(),
        ]).astype(ml_dtypes.bfloat16)
        assert packed.size == IN_ELEMS
        in_maps.append({"inp": packed})
    return in_maps


_NC_CACHE = {}


def get_nc():
    if "nc" not in _NC_CACHE:
        _NC_CACHE["nc"] = build_nc()
    return _NC_CACHE["nc"]


def kernel(x, freqs_cos, freqs_sin, w_attn, w_proj, q_norm_w, k_norm_w):
    nc = get_nc()
    in_maps = make_in_maps(x, freqs_cos, freqs_sin, w_attn, w_proj,
                           q_norm_w, k_norm_w)
    res = run_bass_kernel_spmd(nc, in_maps, core_ids=list(range(N_CORES)))
    # out[c] is [C, 512] feature-major bf16 for tokens [512c, 512c+512)
    cols = [np.asarray(res.results[c]["out"], dtype=np.float32).T
            for c in range(N_CORES)]
    return np.concatenate(cols, axis=0).reshape(B, T, C)
